# revision 31
# baseline (speedup 1.0000x reference)
"""Bilinear (softmax-free) multi-head attention on 8 TRN2 NeuronCores.

Math: for each batch b,
    out_b = x_b @ M_b,   M_b = sum_h Wq[h] @ (Wk[h].T @ (x_b.T x_b) @ Wv[h]) @ Wo[h]
since (Q K^T) V = Q (K^T V) and every projection is linear. This collapses the
O(L^2) attention into two L-sized GEMMs (G = x^T x and out = x @ M) plus a tiny
512x512 head-folding chain.

Distribution (SPMD, no collectives): core i handles batch b = i//4 and output
row chunk c = i%4. Each core streams the full x_b to build G redundantly
(an all-reduce would cost >=15us of collective overhead), folds all 8 heads
into M, and computes/stores only its own 1024-row slice of out.

Precision/perf: the G build and the small chain run in fp8e4m3 with DoubleRow
matmuls (0.5 PE cycles per output row, 256-deep contraction per instruction);
x ships as fp8 (2MB instead of 4MB fp16). Power-of-two scales keep every
intermediate inside fp8/fp16 range: G is scaled 2^-6 at the PSUM->SBUF copy,
and 2^-7/2^-6/2^-5 are folded into Wk/Wo/Wq host-side; the host multiplies the
fp16 output back by 2^24. Wo (the dominant error contributor), the D blocks,
and the final x @ M GEMM stay fp16; out ships fp16. Measured end-to-end rel
err ~6e-3 (limit 2e-2).

Scheduling notes (from TimelineSim traces): PSUM->SBUF copies run only on DVE
and ACT and are the serial bottleneck of the G->B->D->NS->M chain, so G is
computed in FULL (no symmetry triangle: the PE-transpose mirrors cost 6 extra
copies and serialize), stages are split into 256-row contraction waves so each
wave starts after only half the previous stage's copies, and copies alternate
engines in consumption order. The D diag blocks share one psum bank whose
cross-head sub-blocks are overwritten by zero matmuls (DR matmuls reject
tile_position), letting two contiguous half-copies move blockdiag(D^T). DMA
data is usable wire-time+900ns (sem prop); DMAs are batched >=0.22MB (625ns
HWDGE each); out ships as 3 paired DMAs plus 2 short singles. G closes its
four row-blocks staggered (per-block tails over the last 4 DoubleRow tiles)
so the scaled g copies overlap the tails and B's first wave.
"""

import numpy as np
import ml_dtypes

import concourse.tile as tile
from concourse import bacc, mybir
from concourse.bass_utils import run_bass_kernel_spmd

F32 = mybir.dt.float32
F16 = mybir.dt.float16
F8 = mybir.dt.float8e4
E4 = ml_dtypes.float8_e4m3

B, L, D = 2, 4096, 512
H, DK = 8, 64
CHUNK = 1024          # output rows per core
P = 128               # SBUF partitions
TL = L // P           # 32 x-tiles of 128 rows
NDT = TL // 2         # 16 DoubleRow tiles of 256 rows
N_CORES = 8

SG = 2.0 ** -6        # applied at the G psum->sbuf copy
SK = 2.0 ** -7        # folded into W_k
SO = 2.0 ** -6        # folded into W_o
SQ = 2.0 ** -5        # folded into W_q
S_TOT = SG * SK * SO * SQ   # 2^-24; host multiplies the output back

DR = mybir.MatmulPerfMode.DoubleRow

_CACHE = {}


def _build():
    nc = bacc.Bacc("TRN2", target_bir_lowering=False, debug=False)

    x_d = nc.dram_tensor("x", [L, D], F8, kind="ExternalInput").ap()
    xt_d = nc.dram_tensor("xt", [D, CHUNK], F16, kind="ExternalInput").ap()
    wv_d = nc.dram_tensor("wv", [D, D], F8, kind="ExternalInput").ap()    # (d, h*k)
    wk_d = nc.dram_tensor("wk", [D, D], F8, kind="ExternalInput").ap()    # (d, h*k), *SK
    wo_d = nc.dram_tensor("wo", [D, D], F16, kind="ExternalInput").ap()   # (h*k, o), *SO
    wqt_d = nc.dram_tensor("wqt", [D, D], F8, kind="ExternalInput").ap()  # (h*k, d), *SQ
    out_d = nc.dram_tensor("out", [CHUNK, D], F16, kind="ExternalOutput").ap()

    with tile.TileContext(nc) as tc:
        import contextlib

        with contextlib.ExitStack() as ctx:
            wpool = ctx.enter_context(tc.tile_pool(name="wpool", bufs=1))
            xpool = ctx.enter_context(tc.tile_pool(name="xpool", bufs=1))
            spool = ctx.enter_context(tc.tile_pool(name="spool", bufs=1))
            opool = ctx.enter_context(tc.tile_pool(name="opool", bufs=3))
            pg = ctx.enter_context(tc.tile_pool(name="pg", bufs=4, space="PSUM"))
            pt = ctx.enter_context(tc.tile_pool(name="pt", bufs=4, space="PSUM"))

            # Compact D^T holder: head-pair m lives at cols 128m (h0 diag
            # block on partitions 0:64, h1 on 64:128). Filled by ONE psum
            # copy; off-diagonal bytes are exact zeros because the FP psum
            # bank is pre-zeroed by a contraction-1 matmul against zz.
            dtbd = spool.tile([P, 512], F16, tag="dt", name="dtbd")
            zz = spool.tile([P, 512], F8, tag="zz", name="zz")
            nc.gpsimd.memset(zz[:], 0.0)

            # --- x: 8 SBUF tiles of [128, 2048] fp8; tile j holds rows 512j..
            # x_sb[j][p, 512*tt + d] = x[128*(4j+tt) + p, d].
            # 9 DMAs: 2 singles (fast PE start) + 7 doubles (HWDGE is 625ns
            # per DMA, so batch >=0.22MB).
            xr = x_d.rearrange("(t p) d -> p t d", p=P)  # [128, 32, 512]
            x_sb = []
            for j in range(8):
                xt_ = xpool.tile([P, 2048], F8, tag=f"x{j}", name=f"x_sb{j}")
                if j == 0:
                    for hh in range(2):
                        nc.sync.dma_start(
                            out=xt_.rearrange("p (t d) -> p t d", t=4)[:, 2 * hh:2 * hh + 2, :],
                            in_=xr[:, 2 * hh:2 * hh + 2, :],
                        )
                else:
                    nc.sync.dma_start(
                        out=xt_.rearrange("p (t d) -> p t d", t=4),
                        in_=xr[:, j * 4:(j + 1) * 4, :],
                    )
                x_sb.append(xt_)

            # --- weights, ordered by when the chain needs them ---
            wv_sb = wpool.tile([P, 2048], F8, tag="wv", name="wv_sb")
            wk_sb = wpool.tile([P, 2048], F8, tag="wk", name="wk_sb")
            wo_sb = wpool.tile([P, 2048], F16, tag="wo", name="wo_sb")
            wqt_sb = wpool.tile([P, 2048], F8, tag="wqt", name="wqt_sb")
            for sb, dram in ((wv_sb, wv_d), (wk_sb, wk_d), (wo_sb, wo_d), (wqt_sb, wqt_d)):
                nc.sync.dma_start(
                    out=sb.rearrange("p (c j) -> p c j", c=4),
                    in_=dram.rearrange("(c p) j -> p c j", p=P),
                )

            # --- own-chunk x^T (host-transposed): xt_sb[p, 1024*kc + l] ---
            xt_sb = spool.tile([P, 4096], F16, tag="xt", name="xt_sb")
            for h in range(2):
                nc.sync.dma_start(
                    out=xt_sb.rearrange("p (kc l) -> p kc l", kc=4)[:, :, h * 512:(h + 1) * 512],
                    in_=xt_d.rearrange("(kc p) l -> p kc l", p=P)[:, :, h * 512:(h + 1) * 512],
                )

            def xdt(dt):
                # DoubleRow view of x rows 256*dt..256*dt+255: [128, 2, 512]
                j, u = divmod(dt, 2)
                return x_sb[j].rearrange("p (t d) -> p t d", t=4)[:, 2 * u:2 * u + 2, :]

            # Single-engine copies alternated between DVE and ACT (each extra
            # instruction pays a fixed access bubble, so half-splitting loses).
            def cp(eng, dst, src, scale=None):
                if scale is None:
                    if eng == 0:
                        nc.vector.tensor_copy(dst, src)
                    else:
                        nc.scalar.copy(dst, src)
                else:
                    if eng == 0:
                        nc.vector.tensor_scalar_mul(dst, src, scale)
                    else:
                        nc.scalar.mul(dst, src, scale)

            # --- G = x^T x (512x512, fp8 DoubleRow, computed in FULL).
            # t-outer over dts 0..11 while the DMA streams; each row-block m
            # then closes on its own tail over dts 12..15, staggering the four
            # PSUM closes so the scaled g copies overlap the remaining tails.
            g_ps = [pg.tile([P, 512], F32, tag="acc", name=f"g_ps{m}") for m in range(4)]

            def g_mm(m, dt):
                xv = xdt(dt)
                nc.tensor.matmul(
                    g_ps[m][:],
                    lhsT=xv[:, :, m * P:(m + 1) * P],
                    rhs=xv[:, :, 0:512],
                    start=(dt == 0),
                    stop=(dt == NDT - 1),
                    perf_mode=DR,
                )

            for dt in range(NDT - 4):
                for m in range(4):
                    g_mm(m, dt)
            for m in range(4):
                for dt in range(NDT - 4, NDT):
                    g_mm(m, dt)

            g_sb = spool.tile([P, 2048], F8, tag="g", name="g_sb")
            for m in range(4):
                cp(m % 2, g_sb[:, m * 512:(m + 1) * 512], g_ps[m][:], scale=SG)

            def dr_view(sb_tile, kc2, cols):
                return sb_tile.rearrange("p (k c) -> p k c", k=4)[:, 2 * kc2:2 * kc2 + 2, cols]

            # --- B = G @ Wv_all (512x512, fp8 DR), kc2 waves: the first wave
            # needs only g row-blocks 0-1. b_ps lives in the pt pool so the
            # first wave does not wait on g_ps slot reuse. ---
            b_ps = [pt.tile([P, 512], F32, tag="tp", name=f"b_ps{m}") for m in range(4)]
            for kc2 in range(2):
                for m in range(4):
                    nc.tensor.matmul(
                        b_ps[m][:],
                        lhsT=dr_view(g_sb, kc2, slice(m * P, (m + 1) * P)),
                        rhs=dr_view(wv_sb, kc2, slice(0, 512)),
                        start=(kc2 == 0),
                        stop=(kc2 == 1),
                        perf_mode=DR,
                    )
            b_sb = spool.tile([P, 2048], F8, tag="b", name="b_sb")
            for m in range(4):
                cp(m % 2, b_sb[:, m * 512:(m + 1) * 512], b_ps[m][:])

            # --- FP = B^T @ Wk_all in head-pair 128-blocks (fp8 DR, kc2
            # waves) into ONE psum bank; the off-diagonal (cross-head) 64x64
            # sub-blocks are then overwritten with zero matmuls so the bank
            # holds exact blockdiag(D^T) and ONE contiguous copy per column
            # half moves it to SBUF. ---
            fp_ps = pg.tile([P, 512], F32, tag="acc", name="fp_ps")

            def fp_zeros(m):
                nc.tensor.matmul(
                    fp_ps[0:64, m * P + 64:(m + 1) * P],
                    lhsT=zz[0:1, 0:64], rhs=zz[0:1, 64:128],
                    start=True, stop=True,
                )
                nc.tensor.matmul(
                    fp_ps[64:128, m * P:m * P + 64],
                    lhsT=zz[0:1, 0:64], rhs=zz[0:1, 64:128],
                    start=True, stop=True, tile_position=(0, 64),
                )

            for m in range(4):
                nc.tensor.matmul(
                    fp_ps[:, m * P:(m + 1) * P],
                    lhsT=dr_view(b_sb, 0, slice(m * P, (m + 1) * P)),
                    rhs=dr_view(wk_sb, 0, slice(m * P, (m + 1) * P)),
                    start=(m == 0),
                    stop=False,
                    perf_mode=DR,
                )
            # kc2=1 wave runs pairs 2,3 first and closes the group on pair 1,
            # so the zero-fills (which must start after the group stop) for
            # pairs 0,1 — and with them the DVE half-copy — fire earliest.
            for m in (2, 3, 0, 1):
                nc.tensor.matmul(
                    fp_ps[:, m * P:(m + 1) * P],
                    lhsT=dr_view(b_sb, 1, slice(m * P, (m + 1) * P)),
                    rhs=dr_view(wk_sb, 1, slice(m * P, (m + 1) * P)),
                    start=False,
                    stop=(m == 1),
                    perf_mode=DR,
                )
            fp_zeros(0)
            fp_zeros(1)
            fp_zeros(2)
            fp_zeros(3)
            # column halves: N(0)/N(1) read only cols 0:256 (DVE half),
            # N(2)/N(3) the ACT half — both 2D-contiguous, parallel engines
            nc.vector.tensor_copy(dtbd[:, 0:256], fp_ps[:, 0:256])
            nc.scalar.copy(dtbd[:, 256:512], fp_ps[:, 256:512])

            # --- NS = blockdiag(D) @ Wo_stack (fp16): diagonal chunk only ---
            ns_sb = spool.tile([P, 2048], F8, tag="ns", name="ns_sb")
            for m in range(4):
                n_ps = pt.tile([P, 512], F32, tag="tp", name=f"n_ps{m}")
                nc.tensor.matmul(
                    n_ps[:],
                    lhsT=dtbd[:, m * P:(m + 1) * P],
                    rhs=wo_sb[:, m * 512:(m + 1) * 512],
                    start=True,
                    stop=True,
                )
                cp(m % 2, ns_sb[:, m * 512:(m + 1) * 512], n_ps[:])

            # --- M = WqT_stack^T-contract @ NS (fp8 DR), kc2 waves; each
            # m_ps closes on its second-wave matmul, copy follows at once. ---
            m_sb = spool.tile([P, 2048], F16, tag="m", name="m_sb")
            m_ps = [pg.tile([P, 512], F32, tag="acc", name=f"m_ps{m}") for m in range(4)]

            def m_mm(m, kc2):
                nc.tensor.matmul(
                    m_ps[m][:],
                    lhsT=dr_view(wqt_sb, kc2, slice(m * P, (m + 1) * P)),
                    rhs=dr_view(ns_sb, kc2, slice(0, 512)),
                    start=(kc2 == 0),
                    stop=(kc2 == 1),
                    perf_mode=DR,
                )

            for m in range(4):
                m_mm(m, 0)
            for m in range(4):
                m_mm(m, 1)
                cp(m % 2, m_sb[:, m * 512:(m + 1) * 512], m_ps[m][:])

            # --- out chunk = x[c*1024:(c+1)*1024] @ M (fp16). Copies
            # alternate DVE/ACT; rows ship as 3 paired DMAs plus 2 singles at
            # the end (the last block's copy splits across both engines) so
            # the final serial wire time is short. 8 distinct psum banks so
            # no slot-reuse stalls. ---
            def o_mms(lb):
                pool_ = pg if lb < 4 else pt
                tag_ = "acc" if lb < 4 else "tp"
                o_ps = pool_.tile([P, 512], F32, tag=tag_, name=f"o_ps{lb}")
                for kc in range(4):
                    nc.tensor.matmul(
                        o_ps[:],
                        lhsT=xt_sb[:, 1024 * kc + P * lb:1024 * kc + P * (lb + 1)],
                        rhs=m_sb[:, kc * 512:(kc + 1) * 512],
                        start=(kc == 0),
                        stop=(kc == 3),
                    )
                return o_ps

            out_q = out_d.rearrange("(q two p) d -> p q two d", p=P, two=2)
            for pair in range(3):
                o2_sb = opool.tile([P, 1024], F16, tag="o", name=f"o2_{pair}")
                for half in range(2):
                    lb = 2 * pair + half
                    o_ps = o_mms(lb)
                    cp(lb % 2, o2_sb[:, half * 512:(half + 1) * 512], o_ps[:])
                nc.sync.dma_start(
                    out=out_q[:, pair],
                    in_=o2_sb.rearrange("p (two d) -> p two d", two=2),
                )
            o2_sb = opool.tile([P, 1024], F16, tag="o", name="o2_3")
            o_ps = o_mms(6)
            cp(0, o2_sb[:, 0:512], o_ps[:])
            nc.sync.dma_start(out=out_q[:, 3, 0], in_=o2_sb[:, 0:512])
            o_ps = o_mms(7)
            nc.vector.tensor_copy(o2_sb[:, 512:768], o_ps[:, 0:256])
            nc.scalar.copy(o2_sb[:, 768:1024], o_ps[:, 256:512])
            nc.sync.dma_start(out=out_q[:, 3, 1], in_=o2_sb[:, 512:1024])

    nc.compile()
    return nc


def _get_nc():
    if "nc" not in _CACHE:
        _CACHE["nc"] = _build()
    return _CACHE["nc"]


def kernel(x, W_q, W_k, W_v, W_o):
    x = np.ascontiguousarray(np.asarray(x, np.float32))
    W_q = np.asarray(W_q, np.float32)
    W_k = np.asarray(W_k, np.float32)
    W_v = np.asarray(W_v, np.float32)
    W_o = np.asarray(W_o, np.float32)

    wv_all = np.ascontiguousarray(W_v.transpose(1, 0, 2).reshape(D, D)).astype(E4)
    wk_all = np.ascontiguousarray(W_k.transpose(1, 0, 2).reshape(D, D) * SK).astype(E4)
    wqt = np.ascontiguousarray(W_q.transpose(0, 2, 1).reshape(D, D) * SQ).astype(E4)
    wo = np.ascontiguousarray(W_o.reshape(D, D) * SO).astype(np.float16)

    nc = _get_nc()
    x8 = [np.ascontiguousarray(x[b]).astype(E4) for b in range(B)]
    in_maps = []
    for i in range(N_CORES):
        b, c = divmod(i, 4)
        xt = np.ascontiguousarray(x[b, c * CHUNK:(c + 1) * CHUNK].T).astype(np.float16)
        in_maps.append(
            {"x": x8[b], "xt": xt, "wv": wv_all, "wk": wk_all, "wo": wo, "wqt": wqt}
        )

    res = run_bass_kernel_spmd(nc, in_maps, list(range(N_CORES)))

    out = np.empty((B, L, D), np.float32)
    inv = np.float32(1.0 / S_TOT)
    for i in range(N_CORES):
        b, c = divmod(i, 4)
        out[b, c * CHUNK:(c + 1) * CHUNK] = res.results[i]["out"].astype(np.float32) * inv
    return out


# revision 39
# speedup vs baseline: 1.0063x; 1.0063x over previous
"""Bilinear (softmax-free) multi-head attention on 8 TRN2 NeuronCores.

Math: for each batch b,
    out_b = x_b @ M_b,   M_b = sum_h Wq[h] @ (Wk[h].T @ (x_b.T x_b) @ Wv[h]) @ Wo[h]
since (Q K^T) V = Q (K^T V) and every projection is linear. This collapses the
O(L^2) attention into two L-sized GEMMs (G = x^T x and out = x @ M) plus a tiny
512x512 head-folding chain.

Distribution (SPMD, no collectives): core i handles batch b = i//4 and output
row chunk c = i%4. Each core streams the full x_b to build G redundantly
(an all-reduce would cost >=15us of collective overhead), folds all 8 heads
into M, and computes/stores only its own 1024-row slice of out.

Precision/perf: the G build and the small chain run in fp8e4m3 with DoubleRow
matmuls (0.5 PE cycles per output row, 256-deep contraction per instruction);
x ships as fp8 (2MB instead of 4MB fp16). Power-of-two scales keep every
intermediate inside fp8/fp16 range: G is scaled 2^-6 at the PSUM->SBUF copy,
and 2^-7/2^-6/2^-5 are folded into Wk/Wo/Wq host-side; the host multiplies the
fp16 output back by 2^24. Wo (the dominant error contributor), the D blocks,
and the final x @ M GEMM stay fp16; out ships fp16. Measured end-to-end rel
err ~6e-3 (limit 2e-2).

Scheduling notes (from TimelineSim traces): PSUM->SBUF copies run only on DVE
and ACT and are the serial bottleneck of the G->B->D->NS->M chain, so G is
computed in FULL (no symmetry triangle: the PE-transpose mirrors cost 6 extra
copies and serialize), stages are split into 256-row contraction waves so each
wave starts after only half the previous stage's copies, and copies alternate
engines in consumption order. The D diag blocks share one psum bank whose
cross-head sub-blocks are overwritten by zero matmuls (DR matmuls reject
tile_position), letting two contiguous half-copies move blockdiag(D^T). DMA
data is usable wire-time+900ns (sem prop); DMAs are batched >=0.22MB (625ns
HWDGE each); out ships as 3 paired DMAs plus 2 short singles. G closes its
four row-blocks staggered (per-block tails over the last 4 DoubleRow tiles)
so the scaled g copies overlap the tails and B's first wave.
"""

import numpy as np
import ml_dtypes

import concourse.tile as tile
from concourse import bacc, mybir
from concourse.bass_utils import run_bass_kernel_spmd

F32 = mybir.dt.float32
F16 = mybir.dt.float16
F8 = mybir.dt.float8e4
E4 = ml_dtypes.float8_e4m3

B, L, D = 2, 4096, 512
H, DK = 8, 64
CHUNK = 1024          # output rows per core
P = 128               # SBUF partitions
TL = L // P           # 32 x-tiles of 128 rows
NDT = TL // 2         # 16 DoubleRow tiles of 256 rows
N_CORES = 8

SG = 2.0 ** -6        # applied at the G psum->sbuf copy
SK = 2.0 ** -7        # folded into W_k
SO = 2.0 ** -6        # folded into W_o
SQ = 2.0 ** -5        # folded into W_q
S_TOT = SG * SK * SO * SQ   # 2^-24; host multiplies the output back

DR = mybir.MatmulPerfMode.DoubleRow

_CACHE = {}


def _build():
    nc = bacc.Bacc("TRN2", target_bir_lowering=False, debug=False)

    x_d = nc.dram_tensor("x", [L, D], F8, kind="ExternalInput").ap()
    xt_d = nc.dram_tensor("xt", [D, CHUNK], F16, kind="ExternalInput").ap()
    wv_d = nc.dram_tensor("wv", [D, D], F8, kind="ExternalInput").ap()    # (d, h*k)
    wk_d = nc.dram_tensor("wk", [D, D], F8, kind="ExternalInput").ap()    # (d, h*k), *SK
    wo_d = nc.dram_tensor("wo", [D, D], F16, kind="ExternalInput").ap()   # (h*k, o), *SO
    wqt_d = nc.dram_tensor("wqt", [D, D], F8, kind="ExternalInput").ap()  # (h*k, d), *SQ
    out_d = nc.dram_tensor("out", [CHUNK, D], F16, kind="ExternalOutput").ap()

    with tile.TileContext(nc) as tc:
        import contextlib

        with contextlib.ExitStack() as ctx:
            wpool = ctx.enter_context(tc.tile_pool(name="wpool", bufs=1))
            xpool = ctx.enter_context(tc.tile_pool(name="xpool", bufs=1))
            spool = ctx.enter_context(tc.tile_pool(name="spool", bufs=1))
            opool = ctx.enter_context(tc.tile_pool(name="opool", bufs=3))
            pg = ctx.enter_context(tc.tile_pool(name="pg", bufs=4, space="PSUM"))
            pt = ctx.enter_context(tc.tile_pool(name="pt", bufs=4, space="PSUM"))

            # Compact D^T holder: head-pair m lives at cols 128m (h0 diag
            # block on partitions 0:64, h1 on 64:128). Filled by ONE psum
            # copy; off-diagonal bytes are exact zeros because the FP psum
            # bank is pre-zeroed by a contraction-1 matmul against zz.
            dtbd = spool.tile([P, 512], F16, tag="dt", name="dtbd")
            zz = spool.tile([P, 512], F8, tag="zz", name="zz")
            nc.gpsimd.memset(zz[:], 0.0)

            # PE pstate warm-up: a few spaced dummy matmuls on the zero tile
            # start the ramp clock early so the G build runs at full clock
            warm_ps = pt.tile([P, 512], F32, tag="tp", name="warm_ps")
            for w in range(3):
                nc.tensor.matmul(
                    warm_ps[0:64, 0:64],
                    lhsT=zz[0:1, 0:64], rhs=zz[0:1, 64:128],
                    start=True, stop=True,
                )

            # --- x: 8 SBUF tiles of [128, 2048] fp8; tile j holds rows 512j..
            # x_sb[j][p, 512*tt + d] = x[128*(4j+tt) + p, d].
            # 8 uniform 0.25MB DMAs: packing beats "fast-start" singles
            # (HWDGE paces any DMA at ~650ns, so singles stretch the wire by
            # 1.3us for a 364ns earlier PE start).
            xr = x_d.rearrange("(t p) d -> p t d", p=P)  # [128, 32, 512]
            x_sb = []
            for j in range(8):
                xt_ = xpool.tile([P, 2048], F8, tag=f"x{j}", name=f"x_sb{j}")
                nc.sync.dma_start(
                    out=xt_.rearrange("p (t d) -> p t d", t=4),
                    in_=xr[:, j * 4:(j + 1) * 4, :],
                )
                x_sb.append(xt_)

            # --- weights, ordered by when the chain needs them ---
            wv_sb = wpool.tile([P, 2048], F8, tag="wv", name="wv_sb")
            wk_sb = wpool.tile([P, 2048], F8, tag="wk", name="wk_sb")
            wo_sb = wpool.tile([P, 2048], F16, tag="wo", name="wo_sb")
            wqt_sb = wpool.tile([P, 2048], F8, tag="wqt", name="wqt_sb")
            for sb, dram in ((wv_sb, wv_d), (wk_sb, wk_d), (wo_sb, wo_d), (wqt_sb, wqt_d)):
                nc.sync.dma_start(
                    out=sb.rearrange("p (c j) -> p c j", c=4),
                    in_=dram.rearrange("(c p) j -> p c j", p=P),
                )

            # --- own-chunk x^T (host-transposed): xt_sb[p, 1024*kc + l] ---
            xt_sb = spool.tile([P, 4096], F16, tag="xt", name="xt_sb")
            for h in range(2):
                nc.sync.dma_start(
                    out=xt_sb.rearrange("p (kc l) -> p kc l", kc=4)[:, :, h * 512:(h + 1) * 512],
                    in_=xt_d.rearrange("(kc p) l -> p kc l", p=P)[:, :, h * 512:(h + 1) * 512],
                )

            def xdt(dt):
                # DoubleRow view of x rows 256*dt..256*dt+255: [128, 2, 512]
                j, u = divmod(dt, 2)
                return x_sb[j].rearrange("p (t d) -> p t d", t=4)[:, 2 * u:2 * u + 2, :]

            # Single-engine copies alternated between DVE and ACT (each extra
            # instruction pays a fixed access bubble, so half-splitting loses).
            def cp(eng, dst, src, scale=None):
                if scale is None:
                    if eng == 0:
                        nc.vector.tensor_copy(dst, src)
                    else:
                        nc.scalar.copy(dst, src)
                else:
                    if eng == 0:
                        nc.vector.tensor_scalar_mul(dst, src, scale)
                    else:
                        nc.scalar.mul(dst, src, scale)

            # --- G = x^T x (512x512, fp8 DoubleRow, computed in FULL).
            # t-outer over dts 0..11 while the DMA streams; each row-block m
            # then closes on its own tail over dts 12..15, staggering the four
            # PSUM closes so the scaled g copies overlap the remaining tails.
            g_ps = [pg.tile([P, 512], F32, tag="acc", name=f"g_ps{m}") for m in range(4)]

            def g_mm(m, dt):
                xv = xdt(dt)
                nc.tensor.matmul(
                    g_ps[m][:],
                    lhsT=xv[:, :, m * P:(m + 1) * P],
                    rhs=xv[:, :, 0:512],
                    start=(dt == 0),
                    stop=(dt == NDT - 1),
                    perf_mode=DR,
                )

            for dt in range(NDT - 4):
                for m in range(4):
                    g_mm(m, dt)
            for m in range(4):
                for dt in range(NDT - 4, NDT):
                    g_mm(m, dt)

            g_sb = spool.tile([P, 2048], F8, tag="g", name="g_sb")
            for m in range(4):
                cp(m % 2, g_sb[:, m * 512:(m + 1) * 512], g_ps[m][:], scale=SG)

            def dr_view(sb_tile, kc2, cols):
                return sb_tile.rearrange("p (k c) -> p k c", k=4)[:, 2 * kc2:2 * kc2 + 2, cols]

            # --- B = G @ Wv_all (512x512, fp8 DR), kc2 waves: the first wave
            # needs only g row-blocks 0-1. b_ps lives in the pt pool so the
            # first wave does not wait on g_ps slot reuse. ---
            b_ps = [pt.tile([P, 512], F32, tag="tp", name=f"b_ps{m}") for m in range(4)]
            for kc2 in range(2):
                for m in range(4):
                    nc.tensor.matmul(
                        b_ps[m][:],
                        lhsT=dr_view(g_sb, kc2, slice(m * P, (m + 1) * P)),
                        rhs=dr_view(wv_sb, kc2, slice(0, 512)),
                        start=(kc2 == 0),
                        stop=(kc2 == 1),
                        perf_mode=DR,
                    )
            b_sb = spool.tile([P, 2048], F8, tag="b", name="b_sb")
            for m in range(4):
                cp(m % 2, b_sb[:, m * 512:(m + 1) * 512], b_ps[m][:])

            # --- FP = B^T @ Wk_all in head-pair 128-blocks (fp8 DR, kc2
            # waves) into ONE psum bank; the off-diagonal (cross-head) 64x64
            # sub-blocks are then overwritten with zero matmuls so the bank
            # holds exact blockdiag(D^T) and ONE contiguous copy per column
            # half moves it to SBUF. ---
            fp_ps = pg.tile([P, 512], F32, tag="acc", name="fp_ps")

            def fp_zeros(m):
                nc.tensor.matmul(
                    fp_ps[0:64, m * P + 64:(m + 1) * P],
                    lhsT=zz[0:1, 0:64], rhs=zz[0:1, 64:128],
                    start=True, stop=True,
                )
                nc.tensor.matmul(
                    fp_ps[64:128, m * P:m * P + 64],
                    lhsT=zz[0:1, 0:64], rhs=zz[0:1, 64:128],
                    start=True, stop=True, tile_position=(0, 64),
                )

            for m in range(4):
                nc.tensor.matmul(
                    fp_ps[:, m * P:(m + 1) * P],
                    lhsT=dr_view(b_sb, 0, slice(m * P, (m + 1) * P)),
                    rhs=dr_view(wk_sb, 0, slice(m * P, (m + 1) * P)),
                    start=(m == 0),
                    stop=False,
                    perf_mode=DR,
                )
            # kc2=1 wave runs pairs 2,3 first and closes the group on pair 1,
            # so the zero-fills (which must start after the group stop) for
            # pairs 0,1 — and with them the DVE half-copy — fire earliest.
            for m in (2, 3, 0, 1):
                nc.tensor.matmul(
                    fp_ps[:, m * P:(m + 1) * P],
                    lhsT=dr_view(b_sb, 1, slice(m * P, (m + 1) * P)),
                    rhs=dr_view(wk_sb, 1, slice(m * P, (m + 1) * P)),
                    start=False,
                    stop=(m == 1),
                    perf_mode=DR,
                )
            fp_zeros(0)
            fp_zeros(1)
            fp_zeros(2)
            fp_zeros(3)
            # column halves: N(0)/N(1) read only cols 0:256 (DVE half),
            # N(2)/N(3) the ACT half — both 2D-contiguous, parallel engines
            nc.vector.tensor_copy(dtbd[:, 0:256], fp_ps[:, 0:256])
            nc.scalar.copy(dtbd[:, 256:512], fp_ps[:, 256:512])

            # --- NS = blockdiag(D) @ Wo_stack (fp16): diagonal chunk only ---
            ns_sb = spool.tile([P, 2048], F8, tag="ns", name="ns_sb")
            for m in range(4):
                n_ps = pt.tile([P, 512], F32, tag="tp", name=f"n_ps{m}")
                nc.tensor.matmul(
                    n_ps[:],
                    lhsT=dtbd[:, m * P:(m + 1) * P],
                    rhs=wo_sb[:, m * 512:(m + 1) * 512],
                    start=True,
                    stop=True,
                )
                cp(m % 2, ns_sb[:, m * 512:(m + 1) * 512], n_ps[:])

            # --- M = WqT_stack^T-contract @ NS (fp8 DR), kc2 waves; each
            # m_ps closes on its second-wave matmul, copy follows at once. ---
            m_sb = spool.tile([P, 2048], F16, tag="m", name="m_sb")
            m_ps = [pg.tile([P, 512], F32, tag="acc", name=f"m_ps{m}") for m in range(4)]

            def m_mm(m, kc2):
                nc.tensor.matmul(
                    m_ps[m][:],
                    lhsT=dr_view(wqt_sb, kc2, slice(m * P, (m + 1) * P)),
                    rhs=dr_view(ns_sb, kc2, slice(0, 512)),
                    start=(kc2 == 0),
                    stop=(kc2 == 1),
                    perf_mode=DR,
                )

            for m in range(4):
                m_mm(m, 0)
            for m in range(4):
                m_mm(m, 1)
                cp(m % 2, m_sb[:, m * 512:(m + 1) * 512], m_ps[m][:])

            # --- out chunk = x[c*1024:(c+1)*1024] @ M (fp16). Copies
            # alternate DVE/ACT; rows ship as 3 paired DMAs plus 2 singles at
            # the end (the last block's copy splits across both engines) so
            # the final serial wire time is short. 8 distinct psum banks so
            # no slot-reuse stalls. ---
            def o_mms(lb):
                pool_ = pg if lb < 4 else pt
                tag_ = "acc" if lb < 4 else "tp"
                o_ps = pool_.tile([P, 512], F32, tag=tag_, name=f"o_ps{lb}")
                for kc in range(4):
                    nc.tensor.matmul(
                        o_ps[:],
                        lhsT=xt_sb[:, 1024 * kc + P * lb:1024 * kc + P * (lb + 1)],
                        rhs=m_sb[:, kc * 512:(kc + 1) * 512],
                        start=(kc == 0),
                        stop=(kc == 3),
                    )
                return o_ps

            out_q = out_d.rearrange("(q two p) d -> p q two d", p=P, two=2)
            for pair in range(3):
                o2_sb = opool.tile([P, 1024], F16, tag="o", name=f"o2_{pair}")
                for half in range(2):
                    lb = 2 * pair + half
                    o_ps = o_mms(lb)
                    cp(lb % 2, o2_sb[:, half * 512:(half + 1) * 512], o_ps[:])
                nc.sync.dma_start(
                    out=out_q[:, pair],
                    in_=o2_sb.rearrange("p (two d) -> p two d", two=2),
                )
            # last two row blocks: single DMAs with the final block's copy
            # split across both engines, so the serial wire tail is short
            o2_sb = opool.tile([P, 1024], F16, tag="o", name="o2_3")
            o_ps = o_mms(6)
            cp(0, o2_sb[:, 0:512], o_ps[:])
            nc.sync.dma_start(out=out_q[:, 3, 0], in_=o2_sb[:, 0:512])
            o_ps = o_mms(7)
            nc.vector.tensor_copy(o2_sb[:, 512:768], o_ps[:, 0:256])
            nc.scalar.copy(o2_sb[:, 768:1024], o_ps[:, 256:512])
            nc.sync.dma_start(out=out_q[:, 3, 1], in_=o2_sb[:, 512:1024])

    nc.compile()
    return nc


def _get_nc():
    if "nc" not in _CACHE:
        _CACHE["nc"] = _build()
    return _CACHE["nc"]


def kernel(x, W_q, W_k, W_v, W_o):
    x = np.ascontiguousarray(np.asarray(x, np.float32))
    W_q = np.asarray(W_q, np.float32)
    W_k = np.asarray(W_k, np.float32)
    W_v = np.asarray(W_v, np.float32)
    W_o = np.asarray(W_o, np.float32)

    wv_all = np.ascontiguousarray(W_v.transpose(1, 0, 2).reshape(D, D)).astype(E4)
    wk_all = np.ascontiguousarray(W_k.transpose(1, 0, 2).reshape(D, D) * SK).astype(E4)
    wqt = np.ascontiguousarray(W_q.transpose(0, 2, 1).reshape(D, D) * SQ).astype(E4)
    wo = np.ascontiguousarray(W_o.reshape(D, D) * SO).astype(np.float16)

    nc = _get_nc()
    x8 = [np.ascontiguousarray(x[b]).astype(E4) for b in range(B)]
    in_maps = []
    for i in range(N_CORES):
        b, c = divmod(i, 4)
        xt = np.ascontiguousarray(x[b, c * CHUNK:(c + 1) * CHUNK].T).astype(np.float16)
        in_maps.append(
            {"x": x8[b], "xt": xt, "wv": wv_all, "wk": wk_all, "wo": wo, "wqt": wqt}
        )

    res = run_bass_kernel_spmd(nc, in_maps, list(range(N_CORES)))

    out = np.empty((B, L, D), np.float32)
    inv = np.float32(1.0 / S_TOT)
    for i in range(N_CORES):
        b, c = divmod(i, 4)
        out[b, c * CHUNK:(c + 1) * CHUNK] = res.results[i]["out"].astype(np.float32) * inv
    return out


# revision 44
# speedup vs baseline: 1.0112x; 1.0049x over previous
"""Bilinear (softmax-free) multi-head attention on 8 TRN2 NeuronCores.

Math: for each batch b,
    out_b = x_b @ M_b,   M_b = sum_h Wq[h] @ (Wk[h].T @ (x_b.T x_b) @ Wv[h]) @ Wo[h]
since (Q K^T) V = Q (K^T V) and every projection is linear. This collapses the
O(L^2) attention into two L-sized GEMMs (G = x^T x and out = x @ M) plus a tiny
512x512 head-folding chain.

Distribution (SPMD, no collectives): core i handles batch b = i//4 and output
row chunk c = i%4. Each core streams the full x_b to build G redundantly
(an all-reduce would cost >=15us of collective overhead), folds all 8 heads
into M, and computes/stores only its own 1024-row slice of out.

Precision/perf: the G build and the small chain run in fp8e4m3 with DoubleRow
matmuls (0.5 PE cycles per output row, 256-deep contraction per instruction);
x ships as fp8 (2MB instead of 4MB fp16). Power-of-two scales keep every
intermediate inside fp8/fp16 range: G is scaled 2^-6 at the PSUM->SBUF copy,
and 2^-7/2^-6/2^-5 are folded into Wk/Wo/Wq host-side; the host multiplies the
fp16 output back by 2^24. Wo (the dominant error contributor), the D blocks,
and the final x @ M GEMM stay fp16; out ships fp16. Measured end-to-end rel
err ~6e-3 (limit 2e-2).

Scheduling notes (from TimelineSim traces): PSUM->SBUF copies run only on DVE
and ACT and are the serial bottleneck of the G->B->D->NS->M chain, so G is
computed in FULL (no symmetry triangle: the PE-transpose mirrors cost 6 extra
copies and serialize), stages are split into 256-row contraction waves so each
wave starts after only half the previous stage's copies, and copies alternate
engines in consumption order. The D diag blocks share one psum bank whose
cross-head sub-blocks are overwritten by zero matmuls (DR matmuls reject
tile_position), letting two contiguous half-copies move blockdiag(D^T). DMA
data is usable wire-time+900ns (sem prop); DMAs are batched >=0.22MB (625ns
HWDGE each); out ships as 3 paired DMAs plus 2 short singles. G closes its
four row-blocks staggered (per-block tails over the last 4 DoubleRow tiles)
so the scaled g copies overlap the tails and B's first wave.
"""

import numpy as np
import ml_dtypes

import concourse.tile as tile
from concourse import bacc, mybir
from concourse.bass_utils import run_bass_kernel_spmd

F32 = mybir.dt.float32
F16 = mybir.dt.float16
F8 = mybir.dt.float8e4
E4 = ml_dtypes.float8_e4m3

B, L, D = 2, 4096, 512
H, DK = 8, 64
CHUNK = 1024          # output rows per core
P = 128               # SBUF partitions
TL = L // P           # 32 x-tiles of 128 rows
NDT = TL // 2         # 16 DoubleRow tiles of 256 rows
N_CORES = 8

SG = 2.0 ** -6        # applied at the G psum->sbuf copy
SK = 2.0 ** -7        # folded into W_k
SO = 2.0 ** -6        # folded into W_o
SQ = 2.0 ** -5        # folded into W_q
S_TOT = SG * SK * SO * SQ   # 2^-24; host multiplies the output back

DR = mybir.MatmulPerfMode.DoubleRow

_CACHE = {}


def _build():
    nc = bacc.Bacc("TRN2", target_bir_lowering=False, debug=False)

    x_d = nc.dram_tensor("x", [L, D], F8, kind="ExternalInput").ap()
    xt_d = nc.dram_tensor("xt", [D, CHUNK], F16, kind="ExternalInput").ap()
    wv_d = nc.dram_tensor("wv", [D, D], F8, kind="ExternalInput").ap()    # (d, h*k)
    wk_d = nc.dram_tensor("wk", [D, D], F8, kind="ExternalInput").ap()    # (d, h*k), *SK
    wo_d = nc.dram_tensor("wo", [D, D], F16, kind="ExternalInput").ap()   # (h*k, o), *SO
    wqt_d = nc.dram_tensor("wqt", [D, D], F8, kind="ExternalInput").ap()  # (h*k, d), *SQ
    out_d = nc.dram_tensor("out", [CHUNK, D], F16, kind="ExternalOutput").ap()

    with tile.TileContext(nc) as tc:
        import contextlib

        with contextlib.ExitStack() as ctx:
            wpool = ctx.enter_context(tc.tile_pool(name="wpool", bufs=1))
            xpool = ctx.enter_context(tc.tile_pool(name="xpool", bufs=1))
            spool = ctx.enter_context(tc.tile_pool(name="spool", bufs=1))
            opool = ctx.enter_context(tc.tile_pool(name="opool", bufs=3))
            pg = ctx.enter_context(tc.tile_pool(name="pg", bufs=4, space="PSUM"))
            pt = ctx.enter_context(tc.tile_pool(name="pt", bufs=4, space="PSUM"))

            # Compact D^T holder: head-pair m lives at cols 128m (h0 diag
            # block on partitions 0:64, h1 on 64:128). Filled by ONE psum
            # copy; off-diagonal bytes are exact zeros because the FP psum
            # bank is pre-zeroed by a contraction-1 matmul against zz.
            dtbd = spool.tile([P, 512], F16, tag="dt", name="dtbd")
            zz = spool.tile([P, 512], F8, tag="zz", name="zz")
            nc.gpsimd.memset(zz[:], 0.0)

            # PE pstate warm-up: a few spaced dummy matmuls on the zero tile
            # start the ramp clock early so the G build runs at full clock
            warm_ps = pt.tile([P, 512], F32, tag="tp", name="warm_ps")
            for w in range(3):
                nc.tensor.matmul(
                    warm_ps[0:64, 0:64],
                    lhsT=zz[0:1, 0:64], rhs=zz[0:1, 64:128],
                    start=True, stop=True,
                )

            # --- x: 8 SBUF tiles of [128, 2048] fp8; tile j holds rows 512j..
            # x_sb[j][p, 512*tt + d] = x[128*(4j+tt) + p, d].
            # 8 uniform 0.25MB DMAs: packing beats "fast-start" singles
            # (HWDGE paces any DMA at ~650ns, so singles stretch the wire by
            # 1.3us for a 364ns earlier PE start).
            xr = x_d.rearrange("(t p) d -> p t d", p=P)  # [128, 32, 512]
            x_sb = []
            for j in range(8):
                xt_ = xpool.tile([P, 2048], F8, tag=f"x{j}", name=f"x_sb{j}")
                nc.sync.dma_start(
                    out=xt_.rearrange("p (t d) -> p t d", t=4),
                    in_=xr[:, j * 4:(j + 1) * 4, :],
                )
                x_sb.append(xt_)

            # --- weights, ordered by when the chain needs them ---
            wv_sb = wpool.tile([P, 2048], F8, tag="wv", name="wv_sb")
            wk_sb = wpool.tile([P, 2048], F8, tag="wk", name="wk_sb")
            wo_sb = wpool.tile([P, 2048], F16, tag="wo", name="wo_sb")
            wqt_sb = wpool.tile([P, 2048], F8, tag="wqt", name="wqt_sb")
            for sb, dram in ((wv_sb, wv_d), (wk_sb, wk_d), (wo_sb, wo_d), (wqt_sb, wqt_d)):
                nc.sync.dma_start(
                    out=sb.rearrange("p (c j) -> p c j", c=4),
                    in_=dram.rearrange("(c p) j -> p c j", p=P),
                )

            # --- own-chunk x^T (host-transposed): xt_sb[p, 1024*kc + l] ---
            xt_sb = spool.tile([P, 4096], F16, tag="xt", name="xt_sb")
            for h in range(2):
                nc.sync.dma_start(
                    out=xt_sb.rearrange("p (kc l) -> p kc l", kc=4)[:, :, h * 512:(h + 1) * 512],
                    in_=xt_d.rearrange("(kc p) l -> p kc l", p=P)[:, :, h * 512:(h + 1) * 512],
                )

            def xdt(dt):
                # DoubleRow view of x rows 256*dt..256*dt+255: [128, 2, 512]
                j, u = divmod(dt, 2)
                return x_sb[j].rearrange("p (t d) -> p t d", t=4)[:, 2 * u:2 * u + 2, :]

            # Single-engine copies alternated between DVE and ACT (each extra
            # instruction pays a fixed access bubble, so half-splitting loses).
            def cp(eng, dst, src, scale=None):
                if scale is None:
                    if eng == 0:
                        nc.vector.tensor_copy(dst, src)
                    else:
                        nc.scalar.copy(dst, src)
                else:
                    if eng == 0:
                        nc.vector.tensor_scalar_mul(dst, src, scale)
                    else:
                        nc.scalar.mul(dst, src, scale)

            # --- G = x^T x (512x512, fp8 DoubleRow, computed in FULL).
            # t-outer over dts 0..11 while the DMA streams; each row-block m
            # then closes on its own tail over dts 12..15, staggering the four
            # PSUM closes so the scaled g copies overlap the remaining tails.
            g_ps = [pg.tile([P, 512], F32, tag="acc", name=f"g_ps{m}") for m in range(4)]

            def g_mm(m, dt):
                xv = xdt(dt)
                nc.tensor.matmul(
                    g_ps[m][:],
                    lhsT=xv[:, :, m * P:(m + 1) * P],
                    rhs=xv[:, :, 0:512],
                    start=(dt == 0),
                    stop=(dt == NDT - 1),
                    perf_mode=DR,
                )

            for dt in range(NDT - 4):
                for m in range(4):
                    g_mm(m, dt)
            for m in range(4):
                for dt in range(NDT - 4, NDT):
                    g_mm(m, dt)

            g_sb = spool.tile([P, 2048], F8, tag="g", name="g_sb")
            for m in range(2):
                cp(m % 2, g_sb[:, m * 512:(m + 1) * 512], g_ps[m][:], scale=SG)
            for m in range(2, 4):
                # the last two blocks gate B's second wave: halves on both
                # engines shorten their copy latency
                nc.vector.tensor_scalar_mul(
                    g_sb[:, m * 512:m * 512 + 256], g_ps[m][:, 0:256], SG)
                nc.scalar.mul(
                    g_sb[:, m * 512 + 256:(m + 1) * 512], g_ps[m][:, 256:512], SG)

            def dr_view(sb_tile, kc2, cols):
                return sb_tile.rearrange("p (k c) -> p k c", k=4)[:, 2 * kc2:2 * kc2 + 2, cols]

            # --- B = G @ Wv_all (512x512, fp8 DR), kc2 waves: the first wave
            # needs only g row-blocks 0-1. b_ps lives in the pt pool so the
            # first wave does not wait on g_ps slot reuse. ---
            b_ps = [pt.tile([P, 512], F32, tag="tp", name=f"b_ps{m}") for m in range(4)]
            for kc2 in range(2):
                for m in range(4):
                    nc.tensor.matmul(
                        b_ps[m][:],
                        lhsT=dr_view(g_sb, kc2, slice(m * P, (m + 1) * P)),
                        rhs=dr_view(wv_sb, kc2, slice(0, 512)),
                        start=(kc2 == 0),
                        stop=(kc2 == 1),
                        perf_mode=DR,
                    )
            b_sb = spool.tile([P, 2048], F8, tag="b", name="b_sb")
            for m in range(4):
                cp(m % 2, b_sb[:, m * 512:(m + 1) * 512], b_ps[m][:])

            # --- FP = B^T @ Wk_all in head-pair 128-blocks (fp8 DR, kc2
            # waves) into ONE psum bank; the off-diagonal (cross-head) 64x64
            # sub-blocks are then overwritten with zero matmuls so the bank
            # holds exact blockdiag(D^T) and ONE contiguous copy per column
            # half moves it to SBUF. ---
            fp_ps = pg.tile([P, 512], F32, tag="acc", name="fp_ps")

            def fp_zeros(m):
                nc.tensor.matmul(
                    fp_ps[0:64, m * P + 64:(m + 1) * P],
                    lhsT=zz[0:1, 0:64], rhs=zz[0:1, 64:128],
                    start=True, stop=True,
                )
                nc.tensor.matmul(
                    fp_ps[64:128, m * P:m * P + 64],
                    lhsT=zz[0:1, 0:64], rhs=zz[0:1, 64:128],
                    start=True, stop=True, tile_position=(0, 64),
                )

            for m in range(4):
                nc.tensor.matmul(
                    fp_ps[:, m * P:(m + 1) * P],
                    lhsT=dr_view(b_sb, 0, slice(m * P, (m + 1) * P)),
                    rhs=dr_view(wk_sb, 0, slice(m * P, (m + 1) * P)),
                    start=(m == 0),
                    stop=False,
                    perf_mode=DR,
                )
            # kc2=1 wave runs pairs 2,3 first and closes the group on pair 1,
            # so the zero-fills (which must start after the group stop) for
            # pairs 0,1 — and with them the DVE half-copy — fire earliest.
            for m in (2, 3, 0, 1):
                nc.tensor.matmul(
                    fp_ps[:, m * P:(m + 1) * P],
                    lhsT=dr_view(b_sb, 1, slice(m * P, (m + 1) * P)),
                    rhs=dr_view(wk_sb, 1, slice(m * P, (m + 1) * P)),
                    start=False,
                    stop=(m == 1),
                    perf_mode=DR,
                )
            fp_zeros(0)
            fp_zeros(1)
            fp_zeros(2)
            fp_zeros(3)
            # column halves: N(0)/N(1) read only cols 0:256 (DVE half),
            # N(2)/N(3) the ACT half — both 2D-contiguous, parallel engines
            nc.vector.tensor_copy(dtbd[:, 0:256], fp_ps[:, 0:256])
            nc.scalar.copy(dtbd[:, 256:512], fp_ps[:, 256:512])

            # --- NS = blockdiag(D) @ Wo_stack (fp16): diagonal chunk only ---
            ns_sb = spool.tile([P, 2048], F8, tag="ns", name="ns_sb")
            for m in range(4):
                n_ps = pt.tile([P, 512], F32, tag="tp", name=f"n_ps{m}")
                nc.tensor.matmul(
                    n_ps[:],
                    lhsT=dtbd[:, m * P:(m + 1) * P],
                    rhs=wo_sb[:, m * 512:(m + 1) * 512],
                    start=True,
                    stop=True,
                )
                cp(m % 2, ns_sb[:, m * 512:(m + 1) * 512], n_ps[:])

            # --- M = WqT_stack^T-contract @ NS (fp8 DR), kc2 waves; each
            # m_ps closes on its second-wave matmul, copy follows at once. ---
            m_sb = spool.tile([P, 2048], F16, tag="m", name="m_sb")
            m_ps = [pg.tile([P, 512], F32, tag="acc", name=f"m_ps{m}") for m in range(4)]

            def m_mm(m, kc2):
                nc.tensor.matmul(
                    m_ps[m][:],
                    lhsT=dr_view(wqt_sb, kc2, slice(m * P, (m + 1) * P)),
                    rhs=dr_view(ns_sb, kc2, slice(0, 512)),
                    start=(kc2 == 0),
                    stop=(kc2 == 1),
                    perf_mode=DR,
                )

            for m in range(4):
                m_mm(m, 0)
            for m in range(4):
                m_mm(m, 1)
                cp(m % 2, m_sb[:, m * 512:(m + 1) * 512], m_ps[m][:])

            # --- out chunk = x[c*1024:(c+1)*1024] @ M (fp16). Copies
            # alternate DVE/ACT; rows ship as 3 paired DMAs plus 2 singles at
            # the end (the last block's copy splits across both engines) so
            # the final serial wire time is short. 8 distinct psum banks so
            # no slot-reuse stalls. ---
            def o_mms(lb):
                pool_ = pg if lb < 4 else pt
                tag_ = "acc" if lb < 4 else "tp"
                o_ps = pool_.tile([P, 512], F32, tag=tag_, name=f"o_ps{lb}")
                for kc in range(4):
                    nc.tensor.matmul(
                        o_ps[:],
                        lhsT=xt_sb[:, 1024 * kc + P * lb:1024 * kc + P * (lb + 1)],
                        rhs=m_sb[:, kc * 512:(kc + 1) * 512],
                        start=(kc == 0),
                        stop=(kc == 3),
                    )
                return o_ps

            out_q = out_d.rearrange("(q two p) d -> p q two d", p=P, two=2)
            for pair in range(3):
                o2_sb = opool.tile([P, 1024], F16, tag="o", name=f"o2_{pair}")
                for half in range(2):
                    lb = 2 * pair + half
                    o_ps = o_mms(lb)
                    cp(lb % 2, o2_sb[:, half * 512:(half + 1) * 512], o_ps[:])
                nc.sync.dma_start(
                    out=out_q[:, pair],
                    in_=o2_sb.rearrange("p (two d) -> p two d", two=2),
                )
            # last two row blocks: single DMAs with the final block's copy
            # split across both engines, so the serial wire tail is short
            o2_sb = opool.tile([P, 1024], F16, tag="o", name="o2_3")
            o_ps = o_mms(6)
            cp(0, o2_sb[:, 0:512], o_ps[:])
            nc.sync.dma_start(out=out_q[:, 3, 0], in_=o2_sb[:, 0:512])
            o_ps = o_mms(7)
            nc.vector.tensor_copy(o2_sb[:, 512:768], o_ps[:, 0:256])
            nc.scalar.copy(o2_sb[:, 768:1024], o_ps[:, 256:512])
            nc.sync.dma_start(out=out_q[:, 3, 1], in_=o2_sb[:, 512:1024])

    nc.compile()
    return nc


def _get_nc():
    if "nc" not in _CACHE:
        _CACHE["nc"] = _build()
    return _CACHE["nc"]


def kernel(x, W_q, W_k, W_v, W_o):
    x = np.ascontiguousarray(np.asarray(x, np.float32))
    W_q = np.asarray(W_q, np.float32)
    W_k = np.asarray(W_k, np.float32)
    W_v = np.asarray(W_v, np.float32)
    W_o = np.asarray(W_o, np.float32)

    wv_all = np.ascontiguousarray(W_v.transpose(1, 0, 2).reshape(D, D)).astype(E4)
    wk_all = np.ascontiguousarray(W_k.transpose(1, 0, 2).reshape(D, D) * SK).astype(E4)
    wqt = np.ascontiguousarray(W_q.transpose(0, 2, 1).reshape(D, D) * SQ).astype(E4)
    wo = np.ascontiguousarray(W_o.reshape(D, D) * SO).astype(np.float16)

    nc = _get_nc()
    x8 = [np.ascontiguousarray(x[b]).astype(E4) for b in range(B)]
    in_maps = []
    for i in range(N_CORES):
        b, c = divmod(i, 4)
        xt = np.ascontiguousarray(x[b, c * CHUNK:(c + 1) * CHUNK].T).astype(np.float16)
        in_maps.append(
            {"x": x8[b], "xt": xt, "wv": wv_all, "wk": wk_all, "wo": wo, "wqt": wqt}
        )

    res = run_bass_kernel_spmd(nc, in_maps, list(range(N_CORES)))

    out = np.empty((B, L, D), np.float32)
    inv = np.float32(1.0 / S_TOT)
    for i in range(N_CORES):
        b, c = divmod(i, 4)
        out[b, c * CHUNK:(c + 1) * CHUNK] = res.results[i]["out"].astype(np.float32) * inv
    return out
